# revision 2
# baseline (speedup 1.0000x reference)
"""AttentionCropLayer Trainium2 kernel.

Per sample b: offsets (w,h) = floor(clip(locs[b]*224, 44, 180) - 44); output
out[b] = images[b, :, w:w+88, h:h+88] * mask, with mask the fixed 88x88
sigmoid-profile outer product.

The sigmoid profile sig(10*r) - sig(10*(r-88)) is 0.5 at r=0 and within
4.6e-5 of 1.0 for r=1..87, so the mask is approximated by scaling row 0 and
column 0 of each crop by 0.5 (corner by 0.25); interior passes through.
Max relative error ~1e-4, far inside the 2e-2 gate.

Strategy (pure data parallel, 8 cores x 16 samples):
  - host precomputes per-sample flat element offsets (exact same f32 ops as
    the reference) and pads the flat image slab by 64 elements so row reads
    can be padded
  - device, per chunk of 8 samples (=128 partitions, partition = (sample,
    channel)): per sample one dynamic-offset HWDGE read DMA whose descriptors
    are 512B (128-element) padded crop rows -- 512B descriptors stream at
    full DMA rate where 352B exact rows pay the sub-512B latency penalty;
    DVE compacts the 128-stride rows to 88 and applies the edge scales; one
    contiguous 3.96MB SWDGE store per chunk
"""

import sys

if "/opt/trn_rl_repo" not in sys.path:
    sys.path.insert(0, "/opt/trn_rl_repo")

import numpy as np

import concourse.bass as bass
import concourse.bacc as bacc
import concourse.mybir as mybir
from concourse import tile
from concourse.bass_utils import run_bass_kernel_spmd

TL = 44
CROP = 2 * TL          # 88
SCALE = 224.0
B, C, IN = 128, 16, 224
NCORES = 8
BPC = B // NCORES      # 16 samples per core
BLK = 8                # samples per chunk -> BLK*C = 128 partitions
NBLK = BPC // BLK      # 2 chunks per core
FREE = CROP * CROP     # 7744
RWIDE = 128            # padded row read: 128 elems = 512B descriptors
MAXOFF = IN - CROP     # 136
IMSZ = C * IN * IN
CHSZ = IN * IN
PADE = 64              # flat-slab tail padding (>=40 needed by padded rows)
NPAD = BPC * IMSZ + PADE
MAXEOFF = (BPC - 1) * IMSZ + MAXOFF * IN + MAXOFF

_nc_cache = {}


def _build_nc():
    nc = bacc.Bacc(None)
    images = nc.declare_dram_parameter(
        "images", [1, NPAD], mybir.dt.float32, isOutput=False
    )
    offs = nc.declare_dram_parameter(
        "offs", [1, BPC], mybir.dt.int32, isOutput=False
    )
    out = nc.declare_dram_parameter(
        "out", [BPC, C, CROP, CROP], mybir.dt.float32, isOutput=True
    )

    with tile.TileContext(nc) as tc:
        with (
            tc.tile_pool(name="const", bufs=1) as cpool,
            tc.tile_pool(name="work", bufs=1) as wpool,
        ):
            # warm the dynamic-DMA path on both HWDGE rings with a dummy
            # register-offset read: the first dynamic DMA per ring pays a
            # ~10us one-time cold cost (bc-ucode load); absorb it while the
            # offset staging DMA is still in flight
            regs = {}
            for rk, weng in (("sync", nc.sync), ("scalar", nc.scalar)):
                reg = weng.alloc_register(
                    "o_reg_sp" if rk == "sync" else "o_reg_act"
                )
                regs[rk] = reg
                weng.reg_mov(reg, 0)
                ov0 = weng.snap(reg, donate=True, min_val=0, max_val=0)
                wsrc = bass.AP(
                    tensor=images[:].tensor,
                    offset=ov0,
                    ap=[[64, BLK * C], [1, 64]],
                    dep_tracking_offset=0,
                )
                wt_ = cpool.tile([BLK * C, 64], mybir.dt.float32, tag=f"warm_{rk}")
                weng.dma_start(out=wt_[:], in_=wsrc)
            offs_sb = cpool.tile([1, BPC], mybir.dt.int32)
            nc.sync.dma_start(out=offs_sb[:], in_=offs[:])

            engs = {"sync": nc.sync, "scalar": nc.scalar}
            for blk in range(NBLK):
                raw = wpool.tile([BLK * C, CROP * RWIDE], mybir.dt.float32,
                                 tag=f"raw{blk}")
                cmp_ = wpool.tile([BLK * C, FREE], mybir.dt.float32,
                                  tag=f"cmp{blk}")
                for j in range(BLK):
                    s = blk * BLK + j
                    rk = "sync" if j % 2 == 0 else "scalar"
                    eng_, reg_ = engs[rk], regs[rk]
                    eng_.reg_load(reg_, offs_sb[0:1, s : s + 1])
                    ov = eng_.snap(reg_, donate=True, min_val=0, max_val=MAXEOFF)
                    srcap = bass.AP(
                        tensor=images[:].tensor,
                        offset=ov,
                        ap=[[CHSZ, C], [IN, CROP], [1, RWIDE]],
                        dep_tracking_offset=s * IMSZ,
                    )
                    eng_.dma_start(out=raw[j * C : (j + 1) * C, :], in_=srcap)
                # compact 128-stride rows to 88, then scale row 0 / col 0
                rawap = raw[:]
                crop_view = bass.AP(
                    tensor=rawap.tensor,
                    offset=rawap.offset,
                    ap=[rawap.ap[0], [RWIDE, CROP], [1, CROP]],
                )
                nc.vector.tensor_copy(out=cmp_[:], in_=crop_view)
                nc.vector.tensor_scalar_mul(cmp_[:, 0:CROP], cmp_[:, 0:CROP], 0.5)
                cmpap = cmp_[:]
                col_view = bass.AP(
                    tensor=cmpap.tensor,
                    offset=cmpap.offset,
                    ap=[cmpap.ap[0], [CROP, CROP], [1, 1]],
                )
                nc.vector.tensor_scalar_mul(col_view, col_view, 0.5)
                out_view = out[blk * BLK : (blk + 1) * BLK].rearrange(
                    "b c i k -> (b c) (i k)"
                )
                nc.gpsimd.dma_start(out=out_view, in_=cmp_[:])
    nc.finalize()
    return nc


def _get_nc():
    if "nc" not in _nc_cache:
        _nc_cache["nc"] = _build_nc()
    return _nc_cache["nc"]


def _host_offsets(locs):
    locs = np.asarray(locs, dtype=np.float32)
    t = np.clip(locs * np.float32(SCALE), np.float32(TL), np.float32(IN - TL))
    return np.floor(t - np.float32(TL)).astype(np.int32)  # [B, 2] (w, h)


def make_in_maps(images, locs):
    images = np.asarray(images, dtype=np.float32)
    off = _host_offsets(locs)  # [B, 2] (w, h)
    s_idx = np.arange(BPC, dtype=np.int64)
    in_maps = []
    for i in range(NCORES):
        sl = slice(i * BPC, (i + 1) * BPC)
        osh = off[sl].astype(np.int64)
        eoff = (s_idx * IMSZ + osh[:, 0] * IN + osh[:, 1]).astype(np.int32)
        slab = np.zeros((1, NPAD), dtype=np.float32)
        slab[0, : BPC * IMSZ] = images[sl].ravel()
        in_maps.append(
            {
                "images": slab,
                "offs": np.ascontiguousarray(eoff.reshape(1, -1)),
            }
        )
    return in_maps


def run(images, locs, trace=False, **kwargs):
    nc = _get_nc()
    in_maps = make_in_maps(images, locs)
    res = run_bass_kernel_spmd(
        nc, in_maps, core_ids=list(range(NCORES)), trace=trace, **kwargs
    )
    outs = [np.asarray(res.results[i]["out"]) for i in range(NCORES)]
    full = np.concatenate(outs, axis=0).astype(np.float32)
    return full, res


def kernel(images, locs):
    full, _ = run(images, locs, trace=False)
    return full
